# revision 23
# baseline (speedup 1.0000x reference)
"""Trainium2 Bass kernel for the CLC block (grouped 3x3 conv -> BN+ReLU ->
grouped 1x1 conv -> BN+ReLU, twice).

Sharding: pure data parallel, batch 32 -> 4 samples per core on 8 cores.

Per-core design (all f32 storage, float32r matmul views):
  - Channel-major layout: [128 channel partitions, pixels] per 128-channel half.
  - gconv3x3: the torch concat ordering (out o = i*64+g reads inputs 4g..4g+3)
    is made block-diagonal per half by storing gconv OUTPUTS in "g-major"
    order (pos = 4g+i) while gconv INPUTS stay in natural channel order.
    Each tap (dh,dw) is a [K=128,M=128] matmul whose rhs is a shifted window
    of a zero-padded [128, 58*58] input tile; 9 taps accumulate in PSUM.
  - pw 1x1 conv: contracts a full 64-channel block which spans both halves of
    the g-major layout -> 2 accumulating K=128 matmuls per output half.
  - BN + conv-bias fold into the matmul weights (host side); each stage then
    needs a single bias+ReLU pass evacuating PSUM->SBUF (ACT for half 0,
    DVE tensor_scalar add+max for half 1).
"""

import numpy as np

B, C, H, W = 32, 256, 56, 56
EPS = 1e-5
N_CORES = 8
BPC = B // N_CORES  # samples per core
HP, WP = H + 2, W + 2  # padded spatial
NPIX = H * W
NPAD = HP * WP
ROWS_PER_TILE = 7
NT = H // ROWS_PER_TILE  # 8 pixel tiles
TILE_PX = ROWS_PER_TILE * W  # 392
PW_ROWS = 8
NTP = H // PW_ROWS  # 7 pw tiles
PW_PX = PW_ROWS * W  # 448
SWAP_ROWS = H // 2 + 2  # 30 padded rows per swap chunk


# ---------------------------------------------------------------------------
# Host-side weight preparation
# ---------------------------------------------------------------------------

def _bn_fold(bg, bb, bm, bv):
    inv = bg / np.sqrt(bv + EPS)
    return inv, bb - bm * inv  # scale, shift (applied after conv+bias*scale)


def prepare_weights(inp):
    """Returns (wg [128, 2*2*9*128], wp [128, 2*2*2*128], bias [128, 8]) f32.

    wg[k, ((l*2+h)*9 + t)*128 + m]: lhsT for gconv layer l, output half h,
      tap t=3*dh+dw.  k = natural input channel within half h; m = g-major
      output position (g = 32h + m//4, i = m%4).
    wp[k, ((l*2+H)*2 + A)*128 + m]: lhsT for pw layer l, output half H
      (natural order), input half A of the g-major input layout.
    bias[m, s*2 + h]: per-partition bias for stage s in that stage's output
      layout (s=0,2: g-major; s=1,3: natural).
    """
    f32 = np.float32
    wg = np.zeros((128, 2, 2, 9, 128), f32)
    wp = np.zeros((128, 2, 2, 2, 128), f32)
    bias = np.zeros((128, 8), f32)

    gconv_params = [
        (inp["w1"], inp["b1"], inp["bn1a_g"], inp["bn1a_b"], inp["bn1a_m"], inp["bn1a_v"]),
        (inp["w2"], inp["b2"], inp["bn2a_g"], inp["bn2a_b"], inp["bn2a_m"], inp["bn2a_v"]),
    ]
    pw_params = [
        (inp["pw1"], inp["pb1"], inp["bn1b_g"], inp["bn1b_b"], inp["bn1b_m"], inp["bn1b_v"]),
        (inp["pw2"], inp["pb2"], inp["bn2b_g"], inp["bn2b_b"], inp["bn2b_m"], inp["bn2b_v"]),
    ]

    for l, (w, bcv, bg, bb, bm, bv) in enumerate(gconv_params):
        w = np.asarray(w, f32)
        inv, shift = _bn_fold(np.asarray(bg, f32), np.asarray(bb, f32),
                              np.asarray(bm, f32), np.asarray(bv, f32))
        bconv = np.asarray(bcv, f32).reshape(256)  # index i*64+g
        beff = bconv * inv + shift  # natural order o
        for h in range(2):
            for m in range(128):
                g = 32 * h + m // 4
                i = m % 4
                o = i * 64 + g
                for kk in range(4):
                    k = 4 * g + kk - 128 * h
                    for t in range(9):
                        wg[k, l, h, t, m] = w[i, g, kk, t // 3, t % 3] * inv[o]
                bias[m, (2 * l) * 2 + h] = beff[o]

    for l, (w, pb, bg, bb, bm, bv) in enumerate(pw_params):
        w = np.asarray(w, f32).reshape(256, 64)
        inv, shift = _bn_fold(np.asarray(bg, f32), np.asarray(bb, f32),
                              np.asarray(bm, f32), np.asarray(bv, f32))
        beff = np.asarray(pb, f32) * inv + shift
        for Hh in range(2):
            for m in range(128):
                c = 128 * Hh + m
                i = c // 64
                for kap in range(64):
                    p = 4 * kap + i  # g-major position of input channel 64*i+kap
                    A, k = divmod(p, 128)
                    wp[k, l, Hh, A, m] = w[c, kap] * inv[c]
                bias[m, (2 * l + 1) * 2 + Hh] = beff[c]

    if _SPLIT[0]:
        # 64x64 2-slot layout: duplicate the two diagonal 64-blocks onto
        # both partition halves so each row-tile can self-load its weights.
        wg2 = np.zeros_like(wg)
        wg2[0:64, ..., 0:64] = wg[0:64, ..., 0:64]
        wg2[64:128, ..., 0:64] = wg[0:64, ..., 0:64]
        wg2[0:64, ..., 64:128] = wg[64:128, ..., 64:128]
        wg2[64:128, ..., 64:128] = wg[64:128, ..., 64:128]
        wg = wg2
    return (wg.reshape(128, 2 * 2 * 9 * 128).astype(np.float16),
            wp.reshape(128, 2 * 2 * 2 * 128).astype(np.float16),
            bias)


# ---------------------------------------------------------------------------
# Numpy emulation of the exact kernel dataflow (for validation)
# ---------------------------------------------------------------------------

def emulate(inp):
    wg, wp, bias = prepare_weights(inp)
    wg = wg.astype(np.float32).reshape(128, 2, 2, 9, 128)
    wp = wp.astype(np.float32).reshape(128, 2, 2, 2, 128)
    x = np.asarray(inp["x"], np.float32)  # [B, 256, 56, 56]
    out = np.zeros_like(x)

    for n in range(B):
        # natural-order padded input [2][128, 58, 58]
        xpad = np.zeros((2, 128, HP, WP), np.float32)
        for h in range(2):
            xpad[h, :, 1:57, 1:57] = x[n, 128 * h:128 * (h + 1)]

        def gconv(src_pad, l):
            t = [np.zeros((128, H, W), np.float32) for _ in range(2)]
            for h in range(2):
                acc = np.zeros((128, H, W), np.float32)
                for tap in range(9):
                    dh, dw = tap // 3, tap % 3
                    rhs = src_pad[h][:, dh:dh + H, dw:dw + W].reshape(128, -1)
                    acc += (wg[:, l, h, tap, :].T @ rhs).reshape(128, H, W)
                t[h] = np.maximum(acc + bias[:, (2 * l) * 2 + h][:, None, None], 0.0)
            return t  # g-major dense halves

        def pw(tsrc, l):
            dst = [None, None]
            for Hh in range(2):
                acc = np.zeros((128, H * W), np.float32)
                for A in range(2):
                    acc += wp[:, l, Hh, A, :].T @ tsrc[A].reshape(128, -1)
                r = np.maximum(acc + bias[:, (2 * l + 1) * 2 + Hh][:, None], 0.0)
                dst[Hh] = r.reshape(128, H, W)
            return dst  # natural dense halves

        t1 = gconv(xpad, 0)
        t2 = pw(t1, 0)
        t2pad = np.zeros((2, 128, HP, WP), np.float32)
        for h in range(2):
            t2pad[h, :, 1:57, 1:57] = t2[h]
        t3 = gconv(t2pad, 1)
        y = pw(t3, 1)
        out[n, 0:128] = y[0]
        out[n, 128:256] = y[1]
    return out


# ---------------------------------------------------------------------------
# Bass program
# ---------------------------------------------------------------------------

_CACHED = {}
_REPEAT = [1]
_SPLIT = [True]
_DBUF = [True]
_GBUF3 = [True]
_F16IO = [True]
_ABLATE = [frozenset()]  # timing-only experiments: {'gmm','pmm','swap','xio','yio'}
_CHUNK = [True]   # split swap DMAs into row chunks
_PSCFG = [(3, 2)]  # (gconv bufs per tag, pw bufs) PSUM banks: 2*g + p <= 8
_EVAC = ["AV"]  # evac engine rotation: A=ACT, V=DVE (gpsimd cannot read PSUM)
_XDIRECT = [False]  # direct x DMA: tested 266us vs 203us staged - keep off


def set_evac(pat):
    _EVAC[0] = pat


def set_xdirect(v):
    _XDIRECT[0] = bool(v)


def set_pscfg(g, p):
    _PSCFG[0] = (g, p)
_PTMAJOR = [True]  # pw emission order pt-major vs half-major


def set_chunk(v):
    _CHUNK[0] = bool(v)


def set_ptmajor(v):
    _PTMAJOR[0] = bool(v)


def set_f16io(v):
    _F16IO[0] = bool(v)


def set_ablate(*toks):
    _ABLATE[0] = frozenset(toks)


def set_repeat(r):
    _REPEAT[0] = r


def set_split(v):
    _SPLIT[0] = bool(v)


def set_dbuf(v):
    _DBUF[0] = bool(v)


def set_gbuf3(v):
    _GBUF3[0] = bool(v)


def _build_body(tc, y_ap, x_ap, wg_ap, wp_ap, bias_ap, zeros_ap, repeat=1):
    import concourse.bass as bass  # noqa: F401
    from concourse import mybir

    nc = tc.nc
    f32 = mybir.dt.float32
    f16 = mybir.dt.float16
    ADD = mybir.AluOpType.add
    MAX = mybir.AluOpType.max
    RELU = mybir.ActivationFunctionType.Relu

    import contextlib
    ctx = tc._build_ctx  # ExitStack supplied by caller

    const = ctx.enter_context(tc.tile_pool(name="const", bufs=1))
    persist = ctx.enter_context(tc.tile_pool(name="persist", bufs=1))
    gps = ctx.enter_context(tc.tile_pool(name="gps", bufs=_PSCFG[0][0],
                                         space="PSUM"))
    pps = ctx.enter_context(tc.tile_pool(name="pps", bufs=_PSCFG[0][1],
                                         space="PSUM"))

    wg_sb = const.tile([128, 2 * 2 * 9 * 128], f16, tag="wg", name="wg_sb")
    wp_sb = const.tile([128, 2 * 2 * 2 * 128], f16, tag="wp", name="wp_sb")
    bias_sb = const.tile([128, 8], f32, tag="bias", name="bias_sb")
    zeros_sb = const.tile([128, PW_PX], f16, tag="z392", name="zeros_sb")
    nc.sync.dma_start(wg_sb[:], wg_ap)
    nc.sync.dma_start(wp_sb[:], wp_ap)
    nc.sync.dma_start(bias_sb[:], bias_ap)
    nc.sync.dma_start(zeros_sb[:], zeros_ap[:, 0:PW_PX])

    io_dt = f16 if _F16IO[0] else f32
    NB = 2 if _DBUF[0] else 1
    # padded-layout input tiles (borders stay zero forever); xpad double-
    # buffered so sample n+1's load/swap overlap sample n's compute fully
    xpad = [[persist.tile([128, NPAD], f16, tag=f"xpad{b}{h}",
                          name=f"xpad{b}{h}") for h in range(2)]
            for b in range(NB)]
    xstage = [persist.tile([128, NPIX], io_dt, tag=f"xstage{h}",
                           name=f"xstage{h}") for h in range(2)]
    r2pad = [persist.tile([128, NPAD], f16, tag=f"r2pad{h}", name=f"r2pad{h}") for h in range(2)]
    # dense intermediates (t1 reused for t3)
    td = [[persist.tile([128, NPIX], f16, tag=f"td{b}{h}", name=f"td{b}{h}") for h in range(2)] for b in range(NB)]
    swap_for = {}
    if _SPLIT[0]:
        for b in range(NB):
            for h in range(2):
                sx = persist.tile([128, SWAP_ROWS * WP], f16,
                                  tag=f"swx{b}{h}", name=f"swx{b}{h}")
                swap_for[id(xpad[b][h])] = sx
        for h in range(2):
            sr = persist.tile([128, SWAP_ROWS * WP], f16, tag=f"swr{h}",
                              name=f"swr{h}")
            swap_for[id(r2pad[h])] = sr
    ysb = [[persist.tile([128, NPIX], io_dt, tag=f"ysb{b}{h}", name=f"ysb{b}{h}") for h in range(2)] for b in range(NB)]

    def p3(tile_):  # [128, NPAD] -> [128, 58, 58]
        return tile_[:].rearrange("p (a b) -> p a b", b=WP)

    for t in [xp for bb in xpad for xp in bb] + r2pad:
        v = p3(t)
        flat = t[:]
        nc.sync.dma_start(flat[:, 0:WP], zeros_ap[:, 0:WP])
        nc.sync.dma_start(flat[:, (HP - 1) * WP:HP * WP], zeros_ap[:, 0:WP])
        nc.sync.dma_start(v[:, 1:HP - 1, 0:1], zeros_ap[:, 0:HP - 2])
        nc.sync.dma_start(v[:, 1:HP - 1, WP - 1:WP], zeros_ap[:, 0:HP - 2])

    abl = _ABLATE[0]
    gm, pm = 'gmm' in abl, 'pmm' in abl

    evac_i = [0]

    def relu_pass(dst, ps, scol, h):
        # dst = relu(psum + bias[:, scol]); engine rotates per _EVAC pattern
        pat = _EVAC[0]
        e = pat[evac_i[0] % len(pat)]
        evac_i[0] += 1
        if e == "A":
            nc.scalar.activation(dst, ps, RELU, bias=bias_sb[:, scol:scol + 1])
        elif e == "V":
            nc.vector.tensor_scalar(dst, ps, bias_sb[:, scol:scol + 1], 0.0,
                                    op0=ADD, op1=MAX)
        else:
            nfree = dst.free_size()
            if nfree == TILE_PX and len(dst.shape) == 2:
                z = zeros_sb[:, 0:TILE_PX]
            else:
                z = zeros_sb[:].rearrange("p (a b) -> p a b", b=W)[
                    :, 0:ROWS_PER_TILE, :]
            nc.gpsimd.scalar_tensor_tensor(dst, ps, bias_sb[:, scol:scol + 1],
                                           z, ADD, MAX)

    def gconv_stage(src_pads, dst_halves, l):
        if _SPLIT[0]:
            gconv_stage_split(src_pads, dst_halves, l)
            return
        for h in range(2):
            src = p3(src_pads[h])
            for pt in range(NT):
                ps = gps.tile([128, TILE_PX], f32, tag="g", name="psg")
                r0 = pt * ROWS_PER_TILE
                for tap in range(9):
                    if gm and tap > 0:
                        continue
                    dh, dw = tap // 3, tap % 3
                    rhs = src[:, r0 + dh:r0 + dh + ROWS_PER_TILE, dw:dw + W]
                    lhsT = wg_sb[:, ((l * 2 + h) * 9 + tap) * 128:
                                 ((l * 2 + h) * 9 + tap) * 128 + 128]
                    nc.tensor.matmul(ps[:], lhsT=lhsT, rhs=rhs,
                                     start=(tap == 0), stop=(tap == 8 or gm),
                        skip_group_check=True)
                dst = dst_halves[h][:, r0 * W:r0 * W + TILE_PX]
                relu_pass(dst, ps[:], (2 * l) * 2 + h, h)

    def gconv_stage_split(src_pads, dst_halves, l):
        # 64x64 tiling: per (half, row-pair), 4 matmuls/tap land on the 4
        # disjoint PE quadrants (natural + partition-swapped inputs) and run
        # concurrently; two pixel slots accumulate in two PSUM banks.
        NPAIR = NT // 2
        for h in range(2):
            src = p3(src_pads[h])
            swp = swap_for[id(src_pads[h])]
            sw3 = swp[:].rearrange("p (a b) -> p a b", b=WP)
            wbase = ((l * 2 + h) * 9) * 128
            for pp in range(NPAIR):
                ra = pp * ROWS_PER_TILE
                rb = ra + (H // 2)
                psA = gps.tile([128, TILE_PX], f32, tag="g", name="psgA")
                psB = gps.tile([128, TILE_PX], f32, tag="g2", name="psgB")
                for tap in range(9):
                    if gm and tap > 0:
                        continue
                    dh, dw = tap // 3, tap % 3
                    wc = wbase + tap * 128
                    nc.tensor.matmul(
                        psA[0:64, :], lhsT=wg_sb[0:64, wc:wc + 64],
                        rhs=src[0:64, ra + dh:ra + dh + ROWS_PER_TILE, dw:dw + W],
                        start=(tap == 0), stop=(tap == 8 or gm),
                        skip_group_check=True)
                    nc.tensor.matmul(
                        psB[64:128, :], lhsT=wg_sb[64:128, wc + 64:wc + 128],
                        rhs=src[64:128, rb + dh:rb + dh + ROWS_PER_TILE, dw:dw + W],
                        start=(tap == 0), stop=(tap == 8 or gm),
                        skip_group_check=True)
                for tap in range(9):
                    if gm and tap > 0:
                        continue
                    dh, dw = tap // 3, tap % 3
                    wc = wbase + tap * 128
                    nc.tensor.matmul(
                        psA[64:128, :], lhsT=wg_sb[0:64, wc + 64:wc + 128],
                        rhs=sw3[0:64, ra + dh:ra + dh + ROWS_PER_TILE, dw:dw + W],
                        start=(tap == 0), stop=(tap == 8 or gm),
                        skip_group_check=True)
                    nc.tensor.matmul(
                        psB[0:64, :], lhsT=wg_sb[64:128, wc:wc + 64],
                        rhs=sw3[64:128, ra + dh:ra + dh + ROWS_PER_TILE, dw:dw + W],
                        start=(tap == 0), stop=(tap == 8 or gm),
                        skip_group_check=True)
                relu_pass(dst_halves[h][:, ra * W:ra * W + TILE_PX],
                          psA[:], (2 * l) * 2 + h, h)
                relu_pass(dst_halves[h][:, rb * W:rb * W + TILE_PX],
                          psB[:], (2 * l) * 2 + h, (h + 1) % 2)

    def issue_swaps(src_pads):
        # chunked so each transfer fires as soon as its source rows exist
        if not _SPLIT[0]:
            return
        if 'swap' in abl:
            chunks = [(0, 1)]
        elif _CHUNK[0]:
            chunks = [(0, 10), (10, 20), (20, SWAP_ROWS)]
        else:
            chunks = [(0, SWAP_ROWS)]
        for h in range(2):
            t = src_pads[h]
            swp = swap_for[id(t)]
            for r0, r1 in chunks:
                nc.sync.dma_start(swp[0:64, r0 * WP:r1 * WP],
                                  t[64:128, r0 * WP:r1 * WP])
                nc.sync.dma_start(
                    swp[64:128, r0 * WP:r1 * WP],
                    t[0:64, ((H // 2) + r0) * WP:((H // 2) + r1) * WP])

    def pw_stage(src_halves, dst_fn, l):
        # pt-major: both output halves of a row-tile complete together, so
        # downstream swaps/gconv windows unblock earliest
        order = ([(pt, Hh) for pt in range(NT) for Hh in range(2)]
                 if _PTMAJOR[0] else
                 [(pt, Hh) for Hh in range(2) for pt in range(NT)])
        for pt, Hh in order:
            ps = pps.tile([128, TILE_PX], f32, tag="p", name="psp")
            for A in range(2):
                if pm and A > 0:
                    continue
                lhsT = wp_sb[:, ((l * 2 + Hh) * 2 + A) * 128:
                             ((l * 2 + Hh) * 2 + A) * 128 + 128]
                rhs = src_halves[A][:, pt * TILE_PX:(pt + 1) * TILE_PX]
                nc.tensor.matmul(ps[:], lhsT=lhsT, rhs=rhs,
                                 start=(A == 0), stop=(A == 1 or pm))
            dst = dst_fn(Hh, pt)
            relu_pass(dst, ps[:], (2 * l + 1) * 2 + Hh, Hh)

    for rep in range(repeat):
      for n in range(BPC):
        b = n % NB
        xp = xpad[b]
        for h in range(2):
            cs, ce = 128 * h, 128 * (h + 1)
            if _XDIRECT[0] and _F16IO[0]:
                dst3 = p3(xp[h])
                if 'xio' in abl:
                    nc.sync.dma_start(dst3[:, 1:2, 1:57], x_ap[n, cs:ce, 0:1, :])
                else:
                    nc.sync.dma_start(dst3[:, 1:31, 1:57],
                                      x_ap[n, cs:ce, 0:30, :])
                    nc.sync.dma_start(dst3[:, 31:57, 1:57],
                                      x_ap[n, cs:ce, 30:56, :])
            else:
                xext = 64 if 'xio' in abl else NPIX
                nc.sync.dma_start(xstage[h][:, 0:xext],
                                  x_ap[n, cs:ce, 0:xext])
                dst = p3(xp[h])[:, 1:57, 1:57]
                srcv = xstage[h][:].rearrange("p (a b) -> p a b", b=W)
                nc.vector.tensor_copy(dst, srcv)

        tdn = td[b]
        ysn = ysb[b]
        issue_swaps(xp)
        gconv_stage(xp, tdn, 0)

        def r2_dst(Hh, pt):
            return p3(r2pad[Hh])[:, pt * ROWS_PER_TILE + 1:
                                 pt * ROWS_PER_TILE + 1 + ROWS_PER_TILE, 1:57]
        pw_stage(tdn, r2_dst, 0)

        issue_swaps(r2pad)
        gconv_stage(r2pad, tdn, 1)

        def y_dst(Hh, pt):
            return ysn[Hh][:, pt * TILE_PX:(pt + 1) * TILE_PX]
        pw_stage(tdn, y_dst, 1)

        yext = 64 if 'yio' in abl else NPIX
        for h in range(2):
            dst = y_ap[n, 128 * h:128 * (h + 1), 0:yext]
            nc.sync.dma_start(dst, ysn[h][:, 0:yext])


def build_program(repeat=1):
    import contextlib

    import concourse.tile as tile
    from concourse import bacc, mybir

    f32 = mybir.dt.float32
    nc = bacc.Bacc("TRN2", target_bir_lowering=False, debug=False,
                   num_devices=N_CORES)
    f16 = mybir.dt.float16
    io_dt = f16 if _F16IO[0] else f32
    if _XDIRECT[0] and _F16IO[0]:
        x_d = nc.dram_tensor("x", [BPC, C, H, W], io_dt,
                             kind="ExternalInput").ap()
    else:
        x_d = nc.dram_tensor("x", [BPC, C, NPIX], io_dt,
                             kind="ExternalInput").ap()
    wg_d = nc.dram_tensor("wg", [128, 2 * 2 * 9 * 128], f16,
                          kind="ExternalInput").ap()
    wp_d = nc.dram_tensor("wp", [128, 2 * 2 * 2 * 128], f16,
                          kind="ExternalInput").ap()
    bias_d = nc.dram_tensor("bias", [128, 8], f32, kind="ExternalInput").ap()
    zeros_d = nc.dram_tensor("zeros", [128, PW_PX], f16,
                             kind="ExternalInput").ap()
    y_d = nc.dram_tensor("y", [BPC, C, NPIX], io_dt, kind="ExternalOutput").ap()

    with tile.TileContext(nc) as tc:
        with contextlib.ExitStack() as ctx:
            tc._build_ctx = ctx
            _build_body(tc, y_d, x_d, wg_d, wp_d, bias_d, zeros_d, repeat=repeat)
    nc.compile()
    return nc


def prepare_run(inputs):
    """(nc, in_maps) for this config — shared by kernel() and timer.py."""
    wg, wp, bias = prepare_weights(inputs)
    x = np.ascontiguousarray(np.asarray(
        inputs["x"], np.float16 if _F16IO[0] else np.float32))

    key = ("nc", _REPEAT[0], _SPLIT[0], _DBUF[0], _GBUF3[0], _F16IO[0],
           _ABLATE[0], _CHUNK[0], _PTMAJOR[0], _PSCFG[0], _EVAC[0],
           _XDIRECT[0])
    if key not in _CACHED:
        _CACHED[key] = build_program(repeat=_REPEAT[0])
    nc = _CACHED[key]

    xshape = ((BPC, C, H, W) if (_XDIRECT[0] and _F16IO[0])
              else (BPC, C, NPIX))
    in_maps = []
    for i in range(N_CORES):
        in_maps.append({
            "x": x[i * BPC:(i + 1) * BPC].reshape(*xshape),
            "wg": wg, "wp": wp, "bias": bias,
            "zeros": np.zeros((128, PW_PX), np.float16),
        })
    return nc, in_maps


def _run(inputs, trace=False):
    from concourse.bass_utils import run_bass_kernel_spmd

    nc, in_maps = prepare_run(inputs)
    res = run_bass_kernel_spmd(nc, in_maps, list(range(N_CORES)), trace=trace)
    out = np.concatenate(
        [res.results[i]["y"].astype(np.float32).reshape(BPC, C, H, W)
         for i in range(N_CORES)],
        axis=0)
    return out, res


def kernel(**inputs):
    return _run(inputs)[0]



# revision 24
# speedup vs baseline: 1.1745x; 1.1745x over previous
"""Trainium2 Bass kernel for the CLC block (grouped 3x3 conv -> BN+ReLU ->
grouped 1x1 conv -> BN+ReLU, twice).

Sharding: pure data parallel, batch 32 -> 4 samples per core on 8 cores.

Per-core design (all f32 storage, float32r matmul views):
  - Channel-major layout: [128 channel partitions, pixels] per 128-channel half.
  - gconv3x3: the torch concat ordering (out o = i*64+g reads inputs 4g..4g+3)
    is made block-diagonal per half by storing gconv OUTPUTS in "g-major"
    order (pos = 4g+i) while gconv INPUTS stay in natural channel order.
    Each tap (dh,dw) is a [K=128,M=128] matmul whose rhs is a shifted window
    of a zero-padded [128, 58*58] input tile; 9 taps accumulate in PSUM.
  - pw 1x1 conv: contracts a full 64-channel block which spans both halves of
    the g-major layout -> 2 accumulating K=128 matmuls per output half.
  - BN + conv-bias fold into the matmul weights (host side); each stage then
    needs a single bias+ReLU pass evacuating PSUM->SBUF (ACT for half 0,
    DVE tensor_scalar add+max for half 1).
"""

import numpy as np

B, C, H, W = 32, 256, 56, 56
EPS = 1e-5
N_CORES = 8
BPC = B // N_CORES  # samples per core
HP, WP = H + 2, W + 2  # padded spatial
NPIX = H * W
NPAD = HP * WP
ROWS_PER_TILE = 7
NT = H // ROWS_PER_TILE  # 8 pixel tiles
TILE_PX = ROWS_PER_TILE * W  # 392
PW_ROWS = 8
NTP = H // PW_ROWS  # 7 pw tiles
PW_PX = PW_ROWS * W  # 448
SWAP_ROWS = H // 2 + 2  # 30 padded rows per swap chunk


# ---------------------------------------------------------------------------
# Host-side weight preparation
# ---------------------------------------------------------------------------

def _bn_fold(bg, bb, bm, bv):
    inv = bg / np.sqrt(bv + EPS)
    return inv, bb - bm * inv  # scale, shift (applied after conv+bias*scale)


def prepare_weights(inp):
    """Returns (wg [128, 2*2*9*128], wp [128, 2*2*2*128], bias [128, 8]) f32.

    wg[k, ((l*2+h)*9 + t)*128 + m]: lhsT for gconv layer l, output half h,
      tap t=3*dh+dw.  k = natural input channel within half h; m = g-major
      output position (g = 32h + m//4, i = m%4).
    wp[k, ((l*2+H)*2 + A)*128 + m]: lhsT for pw layer l, output half H
      (natural order), input half A of the g-major input layout.
    bias[m, s*2 + h]: per-partition bias for stage s in that stage's output
      layout (s=0,2: g-major; s=1,3: natural).
    """
    f32 = np.float32
    wg = np.zeros((128, 2, 2, 9, 128), f32)
    wp = np.zeros((128, 2, 2, 2, 128), f32)
    bias = np.zeros((128, 8), f32)

    gconv_params = [
        (inp["w1"], inp["b1"], inp["bn1a_g"], inp["bn1a_b"], inp["bn1a_m"], inp["bn1a_v"]),
        (inp["w2"], inp["b2"], inp["bn2a_g"], inp["bn2a_b"], inp["bn2a_m"], inp["bn2a_v"]),
    ]
    pw_params = [
        (inp["pw1"], inp["pb1"], inp["bn1b_g"], inp["bn1b_b"], inp["bn1b_m"], inp["bn1b_v"]),
        (inp["pw2"], inp["pb2"], inp["bn2b_g"], inp["bn2b_b"], inp["bn2b_m"], inp["bn2b_v"]),
    ]

    for l, (w, bcv, bg, bb, bm, bv) in enumerate(gconv_params):
        w = np.asarray(w, f32)
        inv, shift = _bn_fold(np.asarray(bg, f32), np.asarray(bb, f32),
                              np.asarray(bm, f32), np.asarray(bv, f32))
        bconv = np.asarray(bcv, f32).reshape(256)  # index i*64+g
        beff = bconv * inv + shift  # natural order o
        for h in range(2):
            for m in range(128):
                g = 32 * h + m // 4
                i = m % 4
                o = i * 64 + g
                for kk in range(4):
                    k = 4 * g + kk - 128 * h
                    for t in range(9):
                        wg[k, l, h, t, m] = w[i, g, kk, t // 3, t % 3] * inv[o]
                bias[m, (2 * l) * 2 + h] = beff[o]

    for l, (w, pb, bg, bb, bm, bv) in enumerate(pw_params):
        w = np.asarray(w, f32).reshape(256, 64)
        inv, shift = _bn_fold(np.asarray(bg, f32), np.asarray(bb, f32),
                              np.asarray(bm, f32), np.asarray(bv, f32))
        beff = np.asarray(pb, f32) * inv + shift
        for Hh in range(2):
            for m in range(128):
                c = 128 * Hh + m
                i = c // 64
                for kap in range(64):
                    p = 4 * kap + i  # g-major position of input channel 64*i+kap
                    A, k = divmod(p, 128)
                    wp[k, l, Hh, A, m] = w[c, kap] * inv[c]
                bias[m, (2 * l + 1) * 2 + Hh] = beff[c]

    if _SPLIT[0]:
        # 64x64 2-slot layout: duplicate the two diagonal 64-blocks onto
        # both partition halves so each row-tile can self-load its weights.
        wg2 = np.zeros_like(wg)
        wg2[0:64, ..., 0:64] = wg[0:64, ..., 0:64]
        wg2[64:128, ..., 0:64] = wg[0:64, ..., 0:64]
        wg2[0:64, ..., 64:128] = wg[64:128, ..., 64:128]
        wg2[64:128, ..., 64:128] = wg[64:128, ..., 64:128]
        wg = wg2
    return (wg.reshape(128, 2 * 2 * 9 * 128).astype(np.float16),
            wp.reshape(128, 2 * 2 * 2 * 128).astype(np.float16),
            bias)


# ---------------------------------------------------------------------------
# Numpy emulation of the exact kernel dataflow (for validation)
# ---------------------------------------------------------------------------

def emulate(inp):
    wg, wp, bias = prepare_weights(inp)
    wg = wg.astype(np.float32).reshape(128, 2, 2, 9, 128)
    wp = wp.astype(np.float32).reshape(128, 2, 2, 2, 128)
    x = np.asarray(inp["x"], np.float32)  # [B, 256, 56, 56]
    out = np.zeros_like(x)

    for n in range(B):
        # natural-order padded input [2][128, 58, 58]
        xpad = np.zeros((2, 128, HP, WP), np.float32)
        for h in range(2):
            xpad[h, :, 1:57, 1:57] = x[n, 128 * h:128 * (h + 1)]

        def gconv(src_pad, l):
            t = [np.zeros((128, H, W), np.float32) for _ in range(2)]
            for h in range(2):
                acc = np.zeros((128, H, W), np.float32)
                for tap in range(9):
                    dh, dw = tap // 3, tap % 3
                    rhs = src_pad[h][:, dh:dh + H, dw:dw + W].reshape(128, -1)
                    acc += (wg[:, l, h, tap, :].T @ rhs).reshape(128, H, W)
                t[h] = np.maximum(acc + bias[:, (2 * l) * 2 + h][:, None, None], 0.0)
            return t  # g-major dense halves

        def pw(tsrc, l):
            dst = [None, None]
            for Hh in range(2):
                acc = np.zeros((128, H * W), np.float32)
                for A in range(2):
                    acc += wp[:, l, Hh, A, :].T @ tsrc[A].reshape(128, -1)
                r = np.maximum(acc + bias[:, (2 * l + 1) * 2 + Hh][:, None], 0.0)
                dst[Hh] = r.reshape(128, H, W)
            return dst  # natural dense halves

        t1 = gconv(xpad, 0)
        t2 = pw(t1, 0)
        t2pad = np.zeros((2, 128, HP, WP), np.float32)
        for h in range(2):
            t2pad[h, :, 1:57, 1:57] = t2[h]
        t3 = gconv(t2pad, 1)
        y = pw(t3, 1)
        out[n, 0:128] = y[0]
        out[n, 128:256] = y[1]
    return out


# ---------------------------------------------------------------------------
# Bass program
# ---------------------------------------------------------------------------

_CACHED = {}
_REPEAT = [1]
_SPLIT = [True]
_DBUF = [True]
_GBUF3 = [True]
_F16IO = [True]
_ABLATE = [frozenset()]  # timing-only experiments: {'gmm','pmm','swap','xio','yio'}
_CHUNK = [True]   # split swap DMAs into row chunks
_PSCFG = [(3, 2)]  # (gconv bufs per tag, pw bufs) PSUM banks: 2*g + p <= 8
_EVAC = ["AV"]  # evac engine rotation: A=ACT, V=DVE (gpsimd cannot read PSUM)
_XDIRECT = [False]  # direct x DMA: tested 266us vs 203us staged - keep off


def set_evac(pat):
    _EVAC[0] = pat


def set_xdirect(v):
    _XDIRECT[0] = bool(v)


def set_pscfg(g, p):
    _PSCFG[0] = (g, p)
_PTMAJOR = [True]  # pw emission order pt-major vs half-major


def set_chunk(v):
    _CHUNK[0] = bool(v)


def set_ptmajor(v):
    _PTMAJOR[0] = bool(v)


def set_f16io(v):
    _F16IO[0] = bool(v)


def set_ablate(*toks):
    _ABLATE[0] = frozenset(toks)


def set_repeat(r):
    _REPEAT[0] = r


def set_split(v):
    _SPLIT[0] = bool(v)


def set_dbuf(v):
    _DBUF[0] = bool(v)


def set_gbuf3(v):
    _GBUF3[0] = bool(v)


def _build_body(tc, y_ap, x_ap, wg_ap, wp_ap, bias_ap, zeros_ap, repeat=1):
    import concourse.bass as bass  # noqa: F401
    from concourse import mybir

    nc = tc.nc
    f32 = mybir.dt.float32
    f16 = mybir.dt.float16
    ADD = mybir.AluOpType.add
    MAX = mybir.AluOpType.max
    RELU = mybir.ActivationFunctionType.Relu

    import contextlib
    ctx = tc._build_ctx  # ExitStack supplied by caller

    const = ctx.enter_context(tc.tile_pool(name="const", bufs=1))
    persist = ctx.enter_context(tc.tile_pool(name="persist", bufs=1))
    gps = ctx.enter_context(tc.tile_pool(name="gps", bufs=_PSCFG[0][0],
                                         space="PSUM"))
    pps = ctx.enter_context(tc.tile_pool(name="pps", bufs=_PSCFG[0][1],
                                         space="PSUM"))

    wg_sb = const.tile([128, 2 * 2 * 9 * 128], f16, tag="wg", name="wg_sb")
    wp_sb = const.tile([128, 2 * 2 * 2 * 128], f16, tag="wp", name="wp_sb")
    bias_sb = const.tile([128, 8], f32, tag="bias", name="bias_sb")
    zeros_sb = const.tile([128, PW_PX], f16, tag="z392", name="zeros_sb")
    nc.sync.dma_start(wg_sb[:], wg_ap)
    nc.sync.dma_start(wp_sb[:], wp_ap)
    nc.sync.dma_start(bias_sb[:], bias_ap)
    nc.sync.dma_start(zeros_sb[:], zeros_ap[:, 0:PW_PX])

    io_dt = f16 if _F16IO[0] else f32
    NB = 2 if _DBUF[0] else 1
    # padded-layout input tiles (borders stay zero forever); xpad double-
    # buffered so sample n+1's load/swap overlap sample n's compute fully
    xpad = [[persist.tile([128, NPAD], f16, tag=f"xpad{b}{h}",
                          name=f"xpad{b}{h}") for h in range(2)]
            for b in range(NB)]
    xstage = [[persist.tile([128, NPIX], io_dt, tag=f"xstage{b}{h}",
                            name=f"xstage{b}{h}") for h in range(2)]
              for b in range(NB)]
    r2pad = [persist.tile([128, NPAD], f16, tag=f"r2pad{h}", name=f"r2pad{h}") for h in range(2)]
    # dense intermediates (t1 reused for t3)
    td = [[persist.tile([128, NPIX], f16, tag=f"td{b}{h}", name=f"td{b}{h}") for h in range(2)] for b in range(NB)]
    swap_for = {}
    if _SPLIT[0]:
        for b in range(NB):
            for h in range(2):
                sx = persist.tile([128, SWAP_ROWS * WP], f16,
                                  tag=f"swx{b}{h}", name=f"swx{b}{h}")
                swap_for[id(xpad[b][h])] = sx
        for h in range(2):
            sr = persist.tile([128, SWAP_ROWS * WP], f16, tag=f"swr{h}",
                              name=f"swr{h}")
            swap_for[id(r2pad[h])] = sr
    ysb = [[persist.tile([128, NPIX], io_dt, tag=f"ysb{b}{h}", name=f"ysb{b}{h}") for h in range(2)] for b in range(NB)]

    def p3(tile_):  # [128, NPAD] -> [128, 58, 58]
        return tile_[:].rearrange("p (a b) -> p a b", b=WP)

    for t in [xp for bb in xpad for xp in bb] + r2pad:
        v = p3(t)
        flat = t[:]
        nc.sync.dma_start(flat[:, 0:WP], zeros_ap[:, 0:WP])
        nc.sync.dma_start(flat[:, (HP - 1) * WP:HP * WP], zeros_ap[:, 0:WP])
        nc.sync.dma_start(v[:, 1:HP - 1, 0:1], zeros_ap[:, 0:HP - 2])
        nc.sync.dma_start(v[:, 1:HP - 1, WP - 1:WP], zeros_ap[:, 0:HP - 2])

    abl = _ABLATE[0]
    gm, pm = 'gmm' in abl, 'pmm' in abl

    evac_i = [0]

    def relu_pass(dst, ps, scol, h):
        # dst = relu(psum + bias[:, scol]); engine rotates per _EVAC pattern
        pat = _EVAC[0]
        e = pat[evac_i[0] % len(pat)]
        evac_i[0] += 1
        if e == "A":
            nc.scalar.activation(dst, ps, RELU, bias=bias_sb[:, scol:scol + 1])
        elif e == "V":
            nc.vector.tensor_scalar(dst, ps, bias_sb[:, scol:scol + 1], 0.0,
                                    op0=ADD, op1=MAX)
        else:
            nfree = dst.free_size()
            if nfree == TILE_PX and len(dst.shape) == 2:
                z = zeros_sb[:, 0:TILE_PX]
            else:
                z = zeros_sb[:].rearrange("p (a b) -> p a b", b=W)[
                    :, 0:ROWS_PER_TILE, :]
            nc.gpsimd.scalar_tensor_tensor(dst, ps, bias_sb[:, scol:scol + 1],
                                           z, ADD, MAX)

    def gconv_stage(src_pads, dst_halves, l):
        if _SPLIT[0]:
            gconv_stage_split(src_pads, dst_halves, l)
            return
        for h in range(2):
            src = p3(src_pads[h])
            for pt in range(NT):
                ps = gps.tile([128, TILE_PX], f32, tag="g", name="psg")
                r0 = pt * ROWS_PER_TILE
                for tap in range(9):
                    if gm and tap > 0:
                        continue
                    dh, dw = tap // 3, tap % 3
                    rhs = src[:, r0 + dh:r0 + dh + ROWS_PER_TILE, dw:dw + W]
                    lhsT = wg_sb[:, ((l * 2 + h) * 9 + tap) * 128:
                                 ((l * 2 + h) * 9 + tap) * 128 + 128]
                    nc.tensor.matmul(ps[:], lhsT=lhsT, rhs=rhs,
                                     start=(tap == 0), stop=(tap == 8 or gm),
                        skip_group_check=True)
                dst = dst_halves[h][:, r0 * W:r0 * W + TILE_PX]
                relu_pass(dst, ps[:], (2 * l) * 2 + h, h)

    def gconv_stage_split(src_pads, dst_halves, l):
        # 64x64 tiling: per (half, row-pair), 4 matmuls/tap land on the 4
        # disjoint PE quadrants (natural + partition-swapped inputs) and run
        # concurrently; two pixel slots accumulate in two PSUM banks.
        NPAIR = NT // 2
        for h in range(2):
            src = p3(src_pads[h])
            swp = swap_for[id(src_pads[h])]
            sw3 = swp[:].rearrange("p (a b) -> p a b", b=WP)
            wbase = ((l * 2 + h) * 9) * 128
            for pp in range(NPAIR):
                ra = pp * ROWS_PER_TILE
                rb = ra + (H // 2)
                psA = gps.tile([128, TILE_PX], f32, tag="g", name="psgA")
                psB = gps.tile([128, TILE_PX], f32, tag="g2", name="psgB")
                for tap in range(9):
                    if gm and tap > 0:
                        continue
                    dh, dw = tap // 3, tap % 3
                    wc = wbase + tap * 128
                    nc.tensor.matmul(
                        psA[0:64, :], lhsT=wg_sb[0:64, wc:wc + 64],
                        rhs=src[0:64, ra + dh:ra + dh + ROWS_PER_TILE, dw:dw + W],
                        start=(tap == 0), stop=(tap == 8 or gm),
                        skip_group_check=True)
                    nc.tensor.matmul(
                        psB[64:128, :], lhsT=wg_sb[64:128, wc + 64:wc + 128],
                        rhs=src[64:128, rb + dh:rb + dh + ROWS_PER_TILE, dw:dw + W],
                        start=(tap == 0), stop=(tap == 8 or gm),
                        skip_group_check=True)
                for tap in range(9):
                    if gm and tap > 0:
                        continue
                    dh, dw = tap // 3, tap % 3
                    wc = wbase + tap * 128
                    nc.tensor.matmul(
                        psA[64:128, :], lhsT=wg_sb[0:64, wc + 64:wc + 128],
                        rhs=sw3[0:64, ra + dh:ra + dh + ROWS_PER_TILE, dw:dw + W],
                        start=(tap == 0), stop=(tap == 8 or gm),
                        skip_group_check=True)
                    nc.tensor.matmul(
                        psB[0:64, :], lhsT=wg_sb[64:128, wc:wc + 64],
                        rhs=sw3[64:128, ra + dh:ra + dh + ROWS_PER_TILE, dw:dw + W],
                        start=(tap == 0), stop=(tap == 8 or gm),
                        skip_group_check=True)
                relu_pass(dst_halves[h][:, ra * W:ra * W + TILE_PX],
                          psA[:], (2 * l) * 2 + h, h)
                relu_pass(dst_halves[h][:, rb * W:rb * W + TILE_PX],
                          psB[:], (2 * l) * 2 + h, (h + 1) % 2)

    def issue_swaps(src_pads):
        # chunked so each transfer fires as soon as its source rows exist
        if not _SPLIT[0]:
            return
        if 'swap' in abl:
            chunks = [(0, 1)]
        elif _CHUNK[0]:
            chunks = [(0, 10), (10, 20), (20, SWAP_ROWS)]
        else:
            chunks = [(0, SWAP_ROWS)]
        for h in range(2):
            t = src_pads[h]
            swp = swap_for[id(t)]
            for r0, r1 in chunks:
                nc.sync.dma_start(swp[0:64, r0 * WP:r1 * WP],
                                  t[64:128, r0 * WP:r1 * WP])
                nc.sync.dma_start(
                    swp[64:128, r0 * WP:r1 * WP],
                    t[0:64, ((H // 2) + r0) * WP:((H // 2) + r1) * WP])

    def pw_stage(src_halves, dst_fn, l):
        # pt-major: both output halves of a row-tile complete together, so
        # downstream swaps/gconv windows unblock earliest
        order = ([(pt, Hh) for pt in range(NT) for Hh in range(2)]
                 if _PTMAJOR[0] else
                 [(pt, Hh) for Hh in range(2) for pt in range(NT)])
        for pt, Hh in order:
            ps = pps.tile([128, TILE_PX], f32, tag="p", name="psp")
            for A in range(2):
                if pm and A > 0:
                    continue
                lhsT = wp_sb[:, ((l * 2 + Hh) * 2 + A) * 128:
                             ((l * 2 + Hh) * 2 + A) * 128 + 128]
                rhs = src_halves[A][:, pt * TILE_PX:(pt + 1) * TILE_PX]
                nc.tensor.matmul(ps[:], lhsT=lhsT, rhs=rhs,
                                 start=(A == 0), stop=(A == 1 or pm))
            dst = dst_fn(Hh, pt)
            relu_pass(dst, ps[:], (2 * l + 1) * 2 + Hh, Hh)

    def load_x(nidx, b):
        # full x path for sample `nidx` into buffer `b`: HBM -> staging ->
        # padded layout -> partition swaps.  Emitted one sample ahead so the
        # whole chain overlaps the previous sample's compute.
        xpb = xpad[b]
        for h in range(2):
            xext = 64 if 'xio' in abl else NPIX
            nc.sync.dma_start(xstage[b][h][:, 0:xext],
                              x_ap[nidx, 128 * h:128 * (h + 1), 0:xext])
            dst = p3(xpb[h])[:, 1:57, 1:57]
            srcv = xstage[b][h][:].rearrange("p (a b) -> p a b", b=W)
            if h == 0:
                nc.scalar.copy(dst, srcv)
            else:
                nc.vector.tensor_copy(dst, srcv)
        issue_swaps(xpb)

    load_x(0, 0)
    for rep in range(repeat):
      for n in range(BPC):
        b = n % NB
        xp = xpad[b]
        tdn = td[b]
        ysn = ysb[b]
        gconv_stage(xp, tdn, 0)
        load_x((n + 1) % BPC, (n + 1) % NB)  # prefetch next sample's input

        def r2_dst(Hh, pt):
            return p3(r2pad[Hh])[:, pt * ROWS_PER_TILE + 1:
                                 pt * ROWS_PER_TILE + 1 + ROWS_PER_TILE, 1:57]
        pw_stage(tdn, r2_dst, 0)

        issue_swaps(r2pad)
        gconv_stage(r2pad, tdn, 1)

        def y_dst(Hh, pt):
            return ysn[Hh][:, pt * TILE_PX:(pt + 1) * TILE_PX]
        pw_stage(tdn, y_dst, 1)

        yext = 64 if 'yio' in abl else NPIX
        for h in range(2):
            dst = y_ap[n, 128 * h:128 * (h + 1), 0:yext]
            nc.sync.dma_start(dst, ysn[h][:, 0:yext])


def build_program(repeat=1):
    import contextlib

    import concourse.tile as tile
    from concourse import bacc, mybir

    f32 = mybir.dt.float32
    nc = bacc.Bacc("TRN2", target_bir_lowering=False, debug=False,
                   num_devices=N_CORES)
    f16 = mybir.dt.float16
    io_dt = f16 if _F16IO[0] else f32
    if _XDIRECT[0] and _F16IO[0]:
        x_d = nc.dram_tensor("x", [BPC, C, H, W], io_dt,
                             kind="ExternalInput").ap()
    else:
        x_d = nc.dram_tensor("x", [BPC, C, NPIX], io_dt,
                             kind="ExternalInput").ap()
    wg_d = nc.dram_tensor("wg", [128, 2 * 2 * 9 * 128], f16,
                          kind="ExternalInput").ap()
    wp_d = nc.dram_tensor("wp", [128, 2 * 2 * 2 * 128], f16,
                          kind="ExternalInput").ap()
    bias_d = nc.dram_tensor("bias", [128, 8], f32, kind="ExternalInput").ap()
    zeros_d = nc.dram_tensor("zeros", [128, PW_PX], f16,
                             kind="ExternalInput").ap()
    y_d = nc.dram_tensor("y", [BPC, C, NPIX], io_dt, kind="ExternalOutput").ap()

    with tile.TileContext(nc) as tc:
        with contextlib.ExitStack() as ctx:
            tc._build_ctx = ctx
            _build_body(tc, y_d, x_d, wg_d, wp_d, bias_d, zeros_d, repeat=repeat)
    nc.compile()
    return nc


def prepare_run(inputs):
    """(nc, in_maps) for this config — shared by kernel() and timer.py."""
    wg, wp, bias = prepare_weights(inputs)
    x = np.ascontiguousarray(np.asarray(
        inputs["x"], np.float16 if _F16IO[0] else np.float32))

    key = ("nc", _REPEAT[0], _SPLIT[0], _DBUF[0], _GBUF3[0], _F16IO[0],
           _ABLATE[0], _CHUNK[0], _PTMAJOR[0], _PSCFG[0], _EVAC[0],
           _XDIRECT[0])
    if key not in _CACHED:
        _CACHED[key] = build_program(repeat=_REPEAT[0])
    nc = _CACHED[key]

    xshape = ((BPC, C, H, W) if (_XDIRECT[0] and _F16IO[0])
              else (BPC, C, NPIX))
    in_maps = []
    for i in range(N_CORES):
        in_maps.append({
            "x": x[i * BPC:(i + 1) * BPC].reshape(*xshape),
            "wg": wg, "wp": wp, "bias": bias,
            "zeros": np.zeros((128, PW_PX), np.float16),
        })
    return nc, in_maps


def _run(inputs, trace=False):
    from concourse.bass_utils import run_bass_kernel_spmd

    nc, in_maps = prepare_run(inputs)
    res = run_bass_kernel_spmd(nc, in_maps, list(range(N_CORES)), trace=trace)
    out = np.concatenate(
        [res.results[i]["y"].astype(np.float32).reshape(BPC, C, H, W)
         for i in range(N_CORES)],
        axis=0)
    return out, res


def kernel(**inputs):
    return _run(inputs)[0]

